# revision 15
# baseline (speedup 1.0000x reference)
"""Trainium2 Bass kernel for GNN message passing (8 NeuronCores, SPMD).

    out = segment_sum(x[src] @ W, tgt, N) + x @ W_self

Key algebraic identity: segment_sum(x[src] @ W, tgt) = segment_sum(x[src], tgt) @ W,
so the per-edge matmul hoists out of the reduction (21 GFLOP -> 6.6 GFLOP).

Sharding: target nodes are split into 8 contiguous ranges of 12500 (one per
core); edges are bucketed to the core owning their target. Windows of 128
targets are processed in groups of 14; for each (core, group) the host
collects the group's distinct source rows into a contiguous block of a
per-core bf16 source tensor xs (the "halo" for that partition of the graph).

The block is laid out in K-tuples (K=4) of rows, ordered so that K edges of
the same window usually share a storage tuple: one 1KB gather descriptor then
serves K edges. The dma_gather Q7 ucode has ~1us fixed cost per call, a hard
~1024-index cap per call, and ~2.3ns/index of serial descriptor-generation
work, so index count dominates - K-packing divides it by ~K.

Per core, working transposed throughout (out.T = W.T @ hT + W_self.T @ xT),
all in bf16 (rel tolerance 2e-2; bf16 keeps ~3e-3):
  - per group: G[slot, K, f] slab = xs4[idx_slot] via indirect gathers
  - per window (t_w tiles of 128 K-slots):
    S[p, j, K*t+h] = is_equal(tl[p, K*t+h], j) built by DVE in
    [slot-lo, target, half-slot] layout -- all operands 2-byte with packed
    last dim, which qualifies for the DVE 2x_1p fast mode
  - hT (PSUM) += matmul(lhsT=G[:, t, h], rhs=S[:, :, K*t+h])
  - per 4 windows: outT (PSUM) = matmul(lhsT=W, rhs=hT) +
    matmul(lhsT=W_self, rhs=xT_window); ACT copies hT out of PSUM (cast to
    bf16), DVE copies outT (cast to bf16)
The host transposes per-core [128, 12544] bf16 outputs back and concatenates.
"""

import numpy as np

P = 128
D = 128
N_NODES = 100000
N_CORES = 8
N_LOC = N_NODES // N_CORES          # 12500
N_WIN = (N_LOC + P - 1) // P        # 98
N_PAD = N_WIN * P                   # 12544
W_GRP = 7                           # windows per gather group (98 = 14*7)
N_GRP = N_WIN // W_GRP              # 14
WAPPLY = 4                          # windows per W-apply / output DMA group
K = 4                               # rows per gather descriptor

_program_cache: dict = {}


def _build_program(
    layout,
    reps: int = 1,
    n_queues: int = 2,
    gather_split: int = 8,
    dma_scratch: int = 16384,
):
    import concourse.bass as bass
    import concourse.mybir as mybir
    import concourse.tile as tile
    from concourse.bacc import Bacc

    f32 = mybir.dt.float32
    bf16 = mybir.dt.bfloat16

    t = layout["t"]                  # K-slot tiles per window, len 98
    LENK = layout["LENK"]            # row K-tuples per group source block
    T_MAX = layout["T_MAX"]
    t_tot = layout["t_tot"]          # sum(t)
    grp_tiles = [sum(t[g * W_GRP : (g + 1) * W_GRP]) for g in range(N_GRP)]
    TILES_MAX = max(grp_tiles)
    idx_cols32 = layout["idx_cols32"]
    k_const = layout["k_const"]
    tl_cols32 = K * t_tot // 2

    nc = Bacc(num_swdge_queues=n_queues, dynamic_dma_scratch_size=dma_scratch)
    xs_d = nc.declare_dram_parameter("xs", [N_GRP * LENK, K * D], bf16, isOutput=False)
    xT_d = nc.declare_dram_parameter("xT", [D, N_PAD], bf16, isOutput=False)
    iotaB_d = nc.declare_dram_parameter(
        "iotaB", [P, P, K * T_MAX], bf16, isOutput=False
    )
    consts_d = nc.declare_dram_parameter(
        "consts", [P, k_const], mybir.dt.int32, isOutput=False
    )
    outT_d = nc.declare_dram_parameter("outT", [D, N_PAD], bf16, isOutput=True)

    with tile.TileContext(nc) as tc:
        with (
            tc.tile_pool(name="const", bufs=1) as cpool,
            tc.tile_pool(name="gath", bufs=2) as gpool,
            tc.tile_pool(name="spool", bufs=4) as spool,
            tc.tile_pool(name="wtile", bufs=3) as wpool,
            tc.tile_pool(name="psum", bufs=4, space="PSUM") as psum,
            tc.tile_pool(name="opsum", bufs=2, space="PSUM") as opsum,
            tc.tile_pool(name="scratch", bufs=1, space="PSUM") as scratch_pool,
        ):
            scratch_ps = scratch_pool.tile([1, 1], f32)
            const_sb = cpool.tile([P, k_const], mybir.dt.int32)
            nc.sync.dma_start(const_sb[:], consts_d[:])
            iotaB_sb = cpool.tile([P, P, K * T_MAX], bf16)
            nc.sync.dma_start(iotaB_sb[:], iotaB_d[:])
            idx16_sb = const_sb[:, 0:idx_cols32].bitcast(mybir.dt.int16)
            tl_sb = const_sb[:, idx_cols32 : idx_cols32 + tl_cols32].bitcast(bf16)
            w_sb = const_sb[
                :, idx_cols32 + tl_cols32 : idx_cols32 + tl_cols32 + 64
            ].bitcast(bf16)
            ws_sb = const_sb[
                :, idx_cols32 + tl_cols32 + 64 : idx_cols32 + tl_cols32 + 128
            ].bitcast(bf16)

            call_no = 0
            for rep in range(reps):
                toff = 0
                hT_sb = None
                for g in range(N_GRP):
                    TILES_g = grp_tiles[g]
                    G = gpool.tile([P, TILES_MAX, K * D], bf16)
                    ioff = sum(gt * 8 for gt in grp_tiles[:g])  # int16 cols
                    t0 = 0
                    while t0 < TILES_g:
                        tn = min(gather_split or TILES_g, TILES_g - t0)
                        nc.gpsimd.dma_gather(
                            G[:, t0 : t0 + tn, :],
                            xs_d[g * LENK : (g + 1) * LENK, :],
                            idx16_sb[:, ioff + t0 * 8 : ioff + (t0 + tn) * 8],
                            tn * P,
                            tn * P,
                            K * D,
                            queue_num=call_no % n_queues,
                        )
                        t0 += tn
                        call_no += 1
                    goff = 0
                    for wi in range(W_GRP):
                        w = g * W_GRP + wi
                        t_w = t[w]
                        S = spool.tile([P, P, K * T_MAX], bf16)
                        nc.vector.tensor_tensor(
                            out=S[:, :, 0 : K * t_w],
                            in0=tl_sb[:, None, toff : toff + K * t_w].to_broadcast(
                                [P, P, K * t_w]
                            ),
                            in1=iotaB_sb[:, :, 0 : K * t_w],
                            op=mybir.AluOpType.is_equal,
                        )
                        hT_ps = psum.tile([D, P], f32)
                        # bf16 matmuls carry one sync wait; the first real
                        # matmul depends on both S (DVE) and G (gather DMA).
                        # The throwaway matmul makes PE observe the DVE tick
                        # first so each real matmul needs a single wait.
                        nc.tensor.matmul(
                            scratch_ps[:],
                            lhsT=S[:, 0, 0:1],
                            rhs=S[:, 0, 0:1],
                            start=True,
                            stop=True,
                        )
                        for tt in range(t_w):
                            for h in range(K):
                                nc.tensor.matmul(
                                    hT_ps[:],
                                    lhsT=G[:, goff + tt, h * D : (h + 1) * D],
                                    rhs=S[:, :, K * tt + h],
                                    start=(tt == 0 and h == 0),
                                    stop=(tt == t_w - 1 and h == K - 1),
                                )
                        goff += t_w
                        toff += K * t_w
                        # grouped W-apply: stage hT of WAPPLY windows side by
                        # side (ACT copies out of PSUM, casting to bf16), then
                        # stream both weight matmuls at N = WAPPLY*128
                        gi = w % WAPPLY
                        if gi == 0:
                            n_in_grp = min(WAPPLY, N_WIN - w)
                            hT_sb = wpool.tile([D, WAPPLY * P], bf16, tag="hT")
                        nc.scalar.copy(hT_sb[:, gi * P : (gi + 1) * P], hT_ps[:])
                        if gi == n_in_grp - 1:
                            w0 = w - gi
                            span = n_in_grp * P
                            xT_sb = wpool.tile([D, WAPPLY * P], bf16, tag="xT")
                            nc.sync.dma_start(
                                xT_sb[:, :span], xT_d[:, w0 * P : w0 * P + span]
                            )
                            outT_ps = opsum.tile([D, WAPPLY * P], f32)
                            nc.tensor.matmul(
                                outT_ps[:, :span],
                                lhsT=w_sb,
                                rhs=hT_sb[:, :span],
                                start=True,
                                stop=False,
                            )
                            nc.tensor.matmul(
                                outT_ps[:, :span],
                                lhsT=ws_sb,
                                rhs=xT_sb[:, :span],
                                start=False,
                                stop=True,
                            )
                            o_sb = wpool.tile([D, WAPPLY * P], bf16, tag="o")
                            nc.vector.tensor_copy(o_sb[:, :span], outT_ps[:, :span])
                            nc.sync.dma_start(
                                outT_d[:, w0 * P : w0 * P + span], o_sb[:, :span]
                            )

    nc.finalize()
    return nc


def _prep_inputs(x, edge_index, W, W_self):
    """Host-side sharding: bucket+sort edges by target core/window, group
    windows, build per-(core, group) K-tuple-compacted source blocks and
    index/one-hot metadata."""
    import ml_dtypes

    bf16 = ml_dtypes.bfloat16
    x = np.ascontiguousarray(np.asarray(x, dtype=np.float32))
    x_bf = x.astype(bf16)
    W_bf = np.ascontiguousarray(np.asarray(W, dtype=np.float32)).astype(bf16)
    Ws_bf = np.ascontiguousarray(np.asarray(W_self, dtype=np.float32)).astype(bf16)
    ei = np.asarray(edge_index)
    src = ei[0].astype(np.int64)
    tgt = ei[1].astype(np.int64)

    order = np.argsort(tgt, kind="stable")
    src_s = src[order]
    tgt_s = tgt[order]
    core = tgt_s // N_LOC
    wloc = (tgt_s - core * N_LOC) // P
    gw = (core * N_WIN + wloc).astype(np.int64)
    counts = np.bincount(gw, minlength=N_CORES * N_WIN).reshape(N_CORES, N_WIN)
    starts = np.concatenate([[0], np.cumsum(counts.reshape(-1))])

    # Per (core, group): K-tuple the group's sources and build per-window
    # slot lists: (idx int16, tl[K] per slot; -1 = unused half). Single-use
    # sources tuple within their window; multi-use sources tuple with others
    # sharing the same window-set, so every window using the tuple lights up
    # several halves of one slot.
    from collections import defaultdict

    all_slots: list[list[tuple]] = [[None] * N_WIN for _ in range(N_CORES)]
    lenk = np.zeros((N_CORES, N_GRP), np.int64)
    src_blocks: list[list[np.ndarray]] = [[] for _ in range(N_CORES)]
    for c in range(N_CORES):
        for g in range(N_GRP):
            a0 = starts[c * N_WIN + g * W_GRP]
            b0 = starts[c * N_WIN + (g + 1) * W_GRP]
            sg = src_s[a0:b0]
            u, inv = np.unique(sg, return_inverse=True)
            U = len(u)
            cnt = np.bincount(inv, minlength=U)
            ew = gw[a0:b0] - (c * N_WIN + g * W_GRP)  # window-in-group
            single = cnt == 1
            edge_of = np.zeros(U, np.int64)
            edge_of[inv] = np.arange(b0 - a0)
            swin_arr = np.where(single, ew[edge_of], -1)
            tuples: list[np.ndarray] = []
            for wl in range(W_GRP):
                s_w = np.where(swin_arr == wl)[0]
                for i in range(0, len(s_w), K):
                    tuples.append(s_w[i : i + K])
            # multi-use sources grouped by their window-set
            eorder = np.argsort(inv, kind="stable")
            bnd = np.searchsorted(inv[eorder], np.arange(U + 1))
            bys = defaultdict(list)
            for r in np.where(~single)[0]:
                ws = tuple(sorted(set(ew[eorder[bnd[r] : bnd[r + 1]]].tolist())))
                bys[ws].append(r)
            for ranks in bys.values():
                for i in range(0, len(ranks), K):
                    tuples.append(np.asarray(ranks[i : i + K]))
            tuple_id = np.empty(U, np.int64)
            half = np.empty(U, np.int64)
            parts = []
            for ti, tup in enumerate(tuples):
                tuple_id[tup] = ti
                half[tup] = np.arange(len(tup))
                parts.append(
                    np.concatenate([tup, np.repeat(tup[-1:], K - len(tup))])
                )
            order_all = np.concatenate(parts)
            lenk[c, g] = len(tuples)
            src_blocks[c].append(u[order_all])
            for wl in range(W_GRP):
                w = g * W_GRP + wl
                ea = starts[c * N_WIN + w] - a0
                eb = starts[c * N_WIN + w + 1] - a0
                er = inv[ea:eb]
                etl = (tgt_s[a0 + ea : a0 + eb] % N_LOC - w * P).astype(np.float32)
                # group this window's edges by tuple
                per_tup = defaultdict(lambda: defaultdict(list))
                for e in range(len(er)):
                    r = er[e]
                    per_tup[tuple_id[r]][half[r]].append(etl[e])
                idxs = []
                tlh = []
                for ti, halves in per_tup.items():
                    m = max(len(v) for v in halves.values())
                    for j in range(m):
                        vec = [-1.0] * K
                        for h, v in halves.items():
                            if j < len(v):
                                vec[h] = v[j]
                        idxs.append(ti)
                        tlh.append(vec)
                all_slots[c][w] = (
                    np.asarray(idxs, np.int16),
                    np.asarray(tlh, np.float32).reshape(-1, K),
                )

    LENK = int(lenk.max())
    assert LENK <= 32767, LENK

    n_slots = np.array(
        [[len(all_slots[c][w][0]) for w in range(N_WIN)] for c in range(N_CORES)]
    )
    t = np.maximum(1, -(-n_slots.max(axis=0) // P)).astype(np.int64)  # [N_WIN]
    T_MAX = int(t.max())
    t_tot = int(t.sum())

    num_idxs_g = [int(t[g * W_GRP : (g + 1) * W_GRP].sum()) * P for g in range(N_GRP)]
    idx_cols16 = sum(num_idxs_g) // 16
    idx_cols32 = idx_cols16 // 2
    tl_cols32 = K * t_tot // 2
    k_const = idx_cols32 + tl_cols32 + 128

    iotaB = np.broadcast_to(
        np.arange(P, dtype=np.float32).astype(bf16)[None, :, None], (P, P, K * T_MAX)
    ).copy()

    in_maps = []
    for c in range(N_CORES):
        xs = np.zeros((N_GRP * LENK, K * D), bf16)
        idx16 = np.zeros((16, idx_cols16), np.int16)
        tl_img = np.full((P, K * t_tot), -1.0, bf16)
        icol = 0
        toff = 0
        for g in range(N_GRP):
            blk = src_blocks[c][g]  # row ranks into x, len K*lenk[c,g]
            xs[g * LENK : g * LENK + len(blk) // K] = (
                x_bf[blk].reshape(len(blk) // K, K * D)
            )
            call_idx = np.zeros(num_idxs_g[g], np.int16)
            soff = 0
            for wi in range(W_GRP):
                w = g * W_GRP + wi
                idxs, tlh = all_slots[c][w]
                n = len(idxs)
                cap = int(t[w]) * P
                assert n <= cap, (c, w, n, cap)
                call_idx[soff : soff + n] = idxs
                tlz = np.full((cap, K), -1.0, np.float32)
                tlz[:n] = tlh
                # slot s (tile tt=s//128, partition p=s%128), half h
                # -> tl_img[p, toff + K*tt + h]
                for h in range(K):
                    tl_img[:, toff + h : toff + K * int(t[w]) : K] = (
                        tlz[:, h].reshape(int(t[w]), P).T.astype(bf16)
                    )
                soff += cap
                toff += K * int(t[w])
            cols = num_idxs_g[g] // 16
            idx16[:, icol : icol + cols] = call_idx.reshape(cols, 16).T
            icol += cols
        idx_rep = np.tile(idx16, (8, 1))  # replicate across the 8 Q7 stripes
        xT_c = np.zeros((D, N_PAD), bf16)
        xT_c[:, :N_LOC] = x_bf[c * N_LOC : (c + 1) * N_LOC].T
        consts = np.concatenate(
            [
                idx_rep.view(np.int32),
                tl_img.view(np.int32),
                np.ascontiguousarray(W_bf).view(np.int32),
                np.ascontiguousarray(Ws_bf).view(np.int32),
            ],
            axis=1,
        )
        assert consts.shape == (P, k_const), (consts.shape, k_const)
        in_maps.append({"xs": xs, "xT": xT_c, "iotaB": iotaB, "consts": consts})

    layout = {
        "t": [int(v) for v in t],
        "LENK": LENK,
        "T_MAX": T_MAX,
        "t_tot": t_tot,
        "idx_cols32": idx_cols32,
        "k_const": k_const,
    }
    return in_maps, layout


def run(x, edge_index, W, W_self, trace=False, **trace_kwargs):
    """Returns (output [100000,128] float32, BassKernelResults)."""
    from concourse import bass_utils

    in_maps, layout = _prep_inputs(x, edge_index, W, W_self)
    key = tuple(layout["t"]) + (layout["LENK"],)
    nc = _program_cache.get(key)
    if nc is None:
        nc = _build_program(layout)
        _program_cache[key] = nc
    # A NeuronCore occasionally comes up wedged from a previous session
    # (NRT_EXEC_UNIT_UNRECOVERABLE); the failed attempt itself clears it, so
    # one retry recovers.
    try:
        res = bass_utils.run_bass_kernel_spmd(
            nc, in_maps, core_ids=list(range(N_CORES)), trace=trace, **trace_kwargs
        )
    except Exception:
        res = bass_utils.run_bass_kernel_spmd(
            nc, in_maps, core_ids=list(range(N_CORES)), trace=trace, **trace_kwargs
        )
    out = np.empty((N_NODES, D), np.float32)
    for c in range(N_CORES):
        out[c * N_LOC : (c + 1) * N_LOC] = (
            res.results[c]["outT"].astype(np.float32).T[:N_LOC]
        )
    return out, res


def kernel(x, edge_index, W, W_self):
    out, _ = run(x, edge_index, W, W_self, trace=False)
    return out
